# revision 11
# baseline (speedup 1.0000x reference)
"""Trainium2 Bass kernel for nn_Expert_13082470383822.

y = silu(depthwise_causal_conv1d(x, conv_w, K=4) + conv_b);  out = y @ W_proj.T + b_proj
x [4, 4096, 2048] fp32. Data-parallel over the 16384 (batch*seq) tokens across
8 NeuronCores (2048 tokens/core + 3-token halo).

All-bf16 dataflow (x, W, y, out in bf16; PSUM accum fp32), every bulk transfer
fp32-packed (the DMA engines are element-rate limited) and ~1MB-sized (each of
the three DMA rings is a serial FIFO with ~5us per transfer, sharing ~350GB/s
of HBM).

Ring schedule, matched to the PE's in-order W consumption during the chase:
  SP  : x piece0, W pairs 0,3,6, then the output strip-pairs
  ACT : x piece1, W pairs 2,5, then x pieces 2-7 (issues injected into the
        conv stream so the ACT engine never blocks on them)
  GPS : conv consts, W pairs 1,4,7, bias broadcast, final out half
DMA issues wait on semaphore reuse on the ISSUING engine, so compute engines
only carry issues whose semaphores are provably fresh or long-retired.

Work is organized in 256-token pieces (2 matmul strips): conv for piece p+1 is
emitted before piece p's matmuls, sized so the DVE's per-piece load (3 taps x
16 channel tiles + 8 PSUM drains ~ 24.6us) fits inside the PE's 28us per
piece. Piece 0 interleaves its two strips per channel tile (8 matmuls per W
tile) to chase the W stream; later pieces run strip-by-strip with 4-bank PSUM
ping-pong so drains overlap the next strip's matmuls. b_proj is added during
the PSUM->SBUF drain on DVE; outputs leave as fp32-packed bf16 strip-pairs.
"""

import sys

if "/opt/trn_rl_repo" not in sys.path:
    sys.path.insert(0, "/opt/trn_rl_repo")

import numpy as np
import ml_dtypes

B, S, D, KW = 4, 4096, 2048, 4
NCORES = 8
T = (B * S) // NCORES  # tokens per core = 2048
KT = D // 128  # 16 channel tiles
ECH = D // 512  # 4 e-chunks
CW = 256  # conv piece width (tokens)
MS = 128  # matmul strip width (tokens)
NP = T // CW  # 8 conv pieces
XW = KT * (CW + 3)  # flat bf16 elements per x piece row (4144)
NWP = KT // 2  # 8 W pair tiles

_BUILT = {}


def _build_program():
    if "nc" in _BUILT:
        return _BUILT["nc"]

    import concourse.tile as tile
    from concourse import bacc, mybir

    dt = mybir.dt
    AF = mybir.ActivationFunctionType
    ALU = mybir.AluOpType

    nc = bacc.Bacc("TRN2", target_bir_lowering=False, debug=False)
    xs_d = nc.declare_dram_parameter(
        "xs_t", [NP, 128, XW // 2], dt.float32, isOutput=False
    )
    wt = nc.declare_dram_parameter(
        "wt", [NWP, 128, 2, D // 2], dt.float32, isOutput=False
    )
    cw = nc.declare_dram_parameter("cw", [128, KT * KW], dt.float32, isOutput=False)
    cb = nc.declare_dram_parameter("cb", [128, KT], dt.float32, isOutput=False)
    bp = nc.declare_dram_parameter("bp", [1, D], dt.float32, isOutput=False)
    out = nc.declare_dram_parameter(
        "out", [NP, 128, 2, D // 2], dt.float32, isOutput=True
    )

    with tile.TileContext(nc) as tc:
        with (
            tc.tile_pool(name="consts", bufs=1) as cpool,
            tc.tile_pool(name="wpool", bufs=1) as wpool,
            tc.tile_pool(name="xpool", bufs=4) as xpool,
            tc.tile_pool(name="ypool", bufs=3) as ypool,
            tc.tile_pool(name="apool", bufs=4) as apool,
            tc.tile_pool(name="opool", bufs=2) as opool,
            tc.tile_pool(name="pspool", bufs=8, space="PSUM") as pspool,
        ):
            w_sb = [
                wpool.tile([128, 2, D], dt.bfloat16, name=f"w{p}") for p in range(NWP)
            ]

            def w_dma(eng, p):
                eng.dma_start(out=w_sb[p].bitcast(dt.float32), in_=wt[p, :, :, :])

            def x_dma(eng, p, store):
                xt = xpool.tile([128, XW], dt.bfloat16, name="xs", tag="xs")
                eng.dma_start(out=xt.bitcast(dt.float32), in_=xs_d[p, :, :])
                store.append(xt)

            # x pieces 0,1 lead the two HW rings; W pairs follow in
            # consumption-order round-robin across all three rings
            xtiles = []
            x_dma(nc.sync, 0, xtiles)
            x_dma(nc.scalar, 1, xtiles)
            w_dma(nc.scalar, 2)
            w_dma(nc.scalar, 5)
            w_dma(nc.sync, 3)
            w_dma(nc.sync, 6)

            # warm the ACT function table
            dum = cpool.tile([1, 1], dt.float32, name="dum")
            nc.gpsimd.memset(dum[:, :], 0.0)
            nc.scalar.activation(dum[:, :], dum[:, :], AF.Silu, bias=0.0)

            # GPS ring: tiny consts, then the first two W pairs (the chase
            # consumes them first, and both HW rings open with an x piece)
            cw_sb = cpool.tile([128, KT * KW], dt.float32, name="cw_sb")
            nc.gpsimd.dma_start(out=cw_sb[:, :], in_=cw[:, :])
            cb_sb = cpool.tile([128, KT], dt.float32, name="cb_sb")
            nc.gpsimd.dma_start(out=cb_sb[:, :], in_=cb[:, :])
            w_dma(nc.gpsimd, 0)
            w_dma(nc.gpsimd, 1)
            w_dma(nc.gpsimd, 4)
            w_dma(nc.gpsimd, 7)
            bb_sb = cpool.tile([128, D], dt.float32, name="bb_sb")
            nc.gpsimd.dma_start(out=bb_sb[:, :], in_=bp[:, :].broadcast_to([128, D]))

            def w_ap(j, e):
                return w_sb[j // 2][:, j % 2, e * 512 : (e + 1) * 512]

            def emit_conv(p):
                pieces = xtiles[p]
                y = ypool.tile([128, KT, CW], dt.bfloat16, name="ys", tag="ys")
                accs = [None] * KT
                accs[0] = apool.tile([128, CW], dt.bfloat16, name="acc", tag="acc")
                nc.scalar.activation(
                    accs[0][:, :],
                    pieces[:, 0:CW],
                    AF.Copy,
                    bias=0.0,
                    scale=cw_sb[:, 0:1],
                )
                for j in range(KT):
                    base = j * (CW + 3)
                    for k in range(1, KW):
                        nc.vector.scalar_tensor_tensor(
                            accs[j][:, :],
                            pieces[:, base + k : base + k + CW],
                            cw_sb[:, j * KW + k : j * KW + k + 1],
                            accs[j][:, :],
                            ALU.mult,
                            ALU.add,
                        )
                    if j + 1 < KT:
                        accs[j + 1] = apool.tile(
                            [128, CW], dt.bfloat16, name="acc", tag="acc"
                        )
                        nb = (j + 1) * (CW + 3)
                        nc.scalar.activation(
                            accs[j + 1][:, :],
                            pieces[:, nb : nb + CW],
                            AF.Copy,
                            bias=0.0,
                            scale=cw_sb[:, (j + 1) * KW : (j + 1) * KW + 1],
                        )
                    nc.scalar.activation(
                        y[:, j, :],
                        accs[j][:, :],
                        AF.Silu,
                        bias=cb_sb[:, j : j + 1],
                    )
                    if j == 2 and len(xtiles) < NP:
                        # stage the next x piece on the ACT ring mid-conv; the
                        # issue's semaphore is many transfers old, so the ACT
                        # engine never waits here
                        x_dma(nc.scalar, len(xtiles), xtiles)
                return y

            def drain(p, m, pss, os_sb):
                for e in range(ECH):
                    nc.vector.tensor_tensor(
                        out=os_sb[:, m, e * 512 : (e + 1) * 512],
                        in0=pss[e][:, :],
                        in1=bb_sb[:, e * 512 : (e + 1) * 512],
                        op=ALU.add,
                    )
                if p == NP - 1:
                    # final piece: stream strip 0's half out under strip 1's
                    # matmuls, then split strip 1's half across both HW rings
                    if m == 0:
                        nc.sync.dma_start(
                            out=out[p, :, 0, :],
                            in_=os_sb.bitcast(dt.float32)[:, 0, :],
                        )
                    else:
                        nc.sync.dma_start(
                            out=out[p, 0:64, 1, :],
                            in_=os_sb.bitcast(dt.float32)[0:64, 1, :],
                        )
                        nc.scalar.dma_start(
                            out=out[p, 64:128, 1, :],
                            in_=os_sb.bitcast(dt.float32)[64:128, 1, :],
                        )
                elif m == 1:
                    nc.sync.dma_start(
                        out=out[p, :, :, :], in_=os_sb.bitcast(dt.float32)
                    )

            def emit_pe(p, y):
                os_sb = opool.tile([128, 2, D], dt.bfloat16, name="os", tag="os")
                if p == 0:
                    # chase the W stream: both strips per channel tile
                    pss = {
                        m: [
                            pspool.tile([128, 512], dt.float32, name="ps", tag="ps")
                            for _ in range(ECH)
                        ]
                        for m in (0, 1)
                    }
                    for j in range(KT):
                        for m in (0, 1):
                            for e in range(ECH):
                                nc.tensor.matmul(
                                    pss[m][e][:, :],
                                    y[:, j, m * MS : (m + 1) * MS],
                                    w_ap(j, e),
                                    start=(j == 0),
                                    stop=(j == KT - 1),
                                )
                    for m in (0, 1):
                        drain(p, m, pss[m], os_sb)
                else:
                    # strip-sequential with 4-bank ping-pong: strip m's drains
                    # overlap strip m+1's matmuls
                    for m in (0, 1):
                        pss = [
                            pspool.tile([128, 512], dt.float32, name="ps", tag="ps")
                            for _ in range(ECH)
                        ]
                        for j in range(KT):
                            for e in range(ECH):
                                nc.tensor.matmul(
                                    pss[e][:, :],
                                    y[:, j, m * MS : (m + 1) * MS],
                                    w_ap(j, e),
                                    start=(j == 0),
                                    stop=(j == KT - 1),
                                )
                        drain(p, m, pss, os_sb)

            ys = []
            for p in range(NP):
                ys.append(emit_conv(p))
                if p >= 1:
                    emit_pe(p - 1, ys[p - 1])
            emit_pe(NP - 1, ys[NP - 1])

    nc.compile()
    _BUILT["nc"] = nc
    return nc


def _shard_inputs(x, conv_w, conv_b, W_proj, b_proj):
    bf16 = ml_dtypes.bfloat16
    x = np.ascontiguousarray(x, dtype=np.float32)
    wt_np = (
        np.ascontiguousarray(W_proj.T, dtype=np.float32)
        .astype(bf16)
        .reshape(NWP, 2, 128, D)
        .transpose(0, 2, 1, 3)
        .copy()
        .view(np.float32)
    )
    cw_np = np.ascontiguousarray(
        conv_w.reshape(KT, 128, KW).transpose(1, 0, 2).reshape(128, KT * KW),
        dtype=np.float32,
    )
    cb_np = np.ascontiguousarray(conv_b.reshape(KT, 128).T, dtype=np.float32)
    bp_np = np.ascontiguousarray(b_proj.reshape(1, D), dtype=np.float32)

    per_batch = S // T
    in_maps = []
    for c in range(NCORES):
        b = c // per_batch
        s0 = (c % per_batch) * T
        xp = np.zeros((T + 3, D), dtype=np.float32)
        xp[3:] = x[b, s0 : s0 + T]
        if s0 > 0:
            xp[:3] = x[b, s0 - 3 : s0]
        xTc = xp.T  # [D, T+3]
        pieces = np.stack([xTc[:, i * CW : i * CW + CW + 3] for i in range(NP)])
        pieces = pieces.reshape(NP, KT, 128, CW + 3)
        pieces = np.ascontiguousarray(pieces.transpose(0, 2, 1, 3)).reshape(
            NP, 128, XW
        )
        in_maps.append(
            {
                "xs_t": pieces.astype(bf16).view(np.float32),
                "wt": wt_np,
                "cw": cw_np,
                "cb": cb_np,
                "bp": bp_np,
            }
        )
    return in_maps


def run_sharded(x, conv_w, conv_b, W_proj, b_proj, trace=False):
    """Run across the 8 cores; returns (full_out [B,S,D], BassKernelResults)."""
    from concourse.bass_utils import run_bass_kernel_spmd

    nc = _build_program()
    in_maps = _shard_inputs(x, conv_w, conv_b, W_proj, b_proj)
    try:
        res = run_bass_kernel_spmd(nc, in_maps, list(range(NCORES)), trace=trace)
    except Exception:
        # transient device wedges (NRT_EXEC_UNIT_UNRECOVERABLE) clear on retry
        res = run_bass_kernel_spmd(nc, in_maps, list(range(NCORES)), trace=trace)
    full = np.empty((B, S, D), dtype=np.float32)
    per_batch = S // T
    for c in range(NCORES):
        b = c // per_batch
        s0 = (c % per_batch) * T
        o = np.ascontiguousarray(res.results[c]["out"])  # [NP, 128, 2, D//2] f32
        o = (
            o.view(ml_dtypes.bfloat16)
            .reshape(NP, 128, 2, D)
            .transpose(0, 2, 1, 3)
            .reshape(T, D)
            .astype(np.float32)
        )
        full[b, s0 : s0 + T] = o
    return full, res


def kernel(x, conv_w, conv_b, W_proj, b_proj):
    full, _ = run_sharded(x, conv_w, conv_b, W_proj, b_proj, trace=False)
    return full
